# revision 39
# baseline (speedup 1.0000x reference)
"""CTC prefix-score decoder kernel for Trainium2 (8 NeuronCores, SPMD).

Math notes
----------
reference computes, per batch row b and candidate c in ctc_beam_idx[b]:
  logp = log_softmax(x @ W.T + b_bias)          (B,T,V)
  bl_t = logp[:, :, blank];  L_t = cumsum_t(bl)
  xn_t = logp at candidate c
  Pn_t = logaddexp(Pn_{t-1}, L_{t-1}) + xn_t    (Pn_{start-1} = -inf)
  Pb_t = logaddexp(Pn_{t-1}, Pb_{t-1}) + bl_t
  curP = logsumexp_t( logaddexp(Pn_t, Pb_t) ) over valid t
(the `same`-mask branch picks logaddexp(LOGZERO, L) vs L, which are equal
in f32, so it never matters).

Substituting an_t = exp(Pn_t - L_t):
  an_t = r_t * an_{t-1} + r_t,   r_t = exp(xn_t - bl_t) = exp(g_t)
  exp(Pb_t - L_t) = sum_{u<t} an_u
  logaddexp(Pn_t, Pb_t) = L_t + log(S_t),  S_t = sum_{u<=t} an_u
Both an and S are first-order linear recurrences -> hardware
tensor_tensor_scan along the time axis, chunked 4x128 with per-chunk
log-space renormalization (pure gauge offsets q, p) to stay in f32 range.
g is lse-free (the log-softmax normalizer cancels in the ratio), so only
the blank row L needs the lse.

Performance structure (per core = 4 batch rows):
 - big matmul x@W.T in fp8e4m3 DoubleRow (W prescaled by 64; logits are
   64x and get rescaled by the ACT `scale` / later tensor_scalar ops)
 - lse via Exp(accum_out=rowsum) on 1024-wide PSUM tiles; the vocab is
   host-sorted by bias so a per-1024-group constant bias (ACT bias slot)
   replaces the exact per-column bias; sel/blank columns stay exact, so
   only lse carries the tiny (~1e-3 abs on L) grouping error.
 - candidate+blank columns via a small fp8 matmul against gathered W rows,
   PE-transposed into a [128 = 4b x 32, T] row layout (32-row alignment).
 - phase 2 (scans, L cumsum, masked logsumexp) is chunked over t and
   interleaved with phase 1 so it hides under the matmul/exp stream.
"""

import functools
import sys

import numpy as np

sys.path.insert(0, "/opt/trn_rl_repo")

import concourse.mybir as mybir  # noqa: E402
from concourse import bacc, bass_utils  # noqa: E402
from concourse.masks import make_identity  # noqa: E402
from concourse.tile import TileContext  # noqa: E402

import ml_dtypes  # noqa: E402

LOGZERO = -(65504.0 ** 2)
B, T, D, V, CB = 32, 512, 512, 4096, 30
NB = B // 8          # batch rows per core
ROWS = 128           # padded scan rows per core (32 per batch row)
KD = D // 128        # 4 contraction sub-chunks of 128
NEG = -60000.0       # "log zero" that exps to exactly 0.0 in f32
WSCALE = 64.0        # fp8 weight prescale
F32 = mybir.dt.float32
BF16 = mybir.dt.bfloat16
FP8 = mybir.dt.float8e4
AX = mybir.AxisListType.X
OP = mybir.AluOpType
AF = mybir.ActivationFunctionType
DR = mybir.MatmulPerfMode.DoubleRow
I32 = mybir.dt.int32
SCH_A = 2.0 ** 23 / np.log(2.0)      # Schraudolph exp constant
SCH_B = 1064866805.0                 # bias incl. min-RMS correction
BF16NP = ml_dtypes.bfloat16
FP8NP = mybir.dt.np(mybir.dt.float8e4)


def _patch_act_tables():
    """Only Exp/Ln (+copy-family) are used; making the combined
    natural_log_exp set the only Exp/Ln provider stops walrus from
    thrashing ACT table loads (~2.7us each)."""
    import concourse.hw_specs as hw_specs

    orig = hw_specs.get_activation_tables

    def filtered(module_arch):
        tabs = orig(module_arch)
        if "natural_log_exp_and_others" not in tabs:
            return tabs
        drop = {AF.Exp, AF.Ln}
        return {
            k: (v if k == "natural_log_exp_and_others" else v - drop)
            for k, v in tabs.items()
        }

    bacc.get_activation_tables = filtered


_patch_act_tables()


@functools.lru_cache(maxsize=4)
def _build(variant=""):
    nc = bacc.Bacc("TRN2", target_bir_lowering=False, debug=False, num_devices=8)

    xT_d = nc.dram_tensor("xT", [NB, 128, KD, T], FP8, kind="ExternalInput").ap()
    WT_d = nc.dram_tensor("WT", [128, KD, V], FP8, kind="ExternalInput").ap()
    Ws_d = nc.dram_tensor("WselT", [NB, 128, KD, CB + 1], FP8,
                          kind="ExternalInput").ap()
    bg_d = nc.dram_tensor("bgs", [128, 4], F32, kind="ExternalInput").ap()
    b2_d = nc.dram_tensor("bgs2", [128, 4], F32, kind="ExternalInput").ap()
    bs_d = nc.dram_tensor("bsel", [ROWS, 1], F32, kind="ExternalInput").ap()
    mk_d = nc.dram_tensor("mask4", [NB, T], F32, kind="ExternalInput").ap()
    in_d = nc.dram_tensor("ind", [NB, ROWS], F32, kind="ExternalInput").ap()
    cp_d = nc.dram_tensor("curP", [ROWS, 1], F32, kind="ExternalOutput").ap()
    L_d = nc.dram_tensor("L", [NB, T], F32, kind="ExternalOutput").ap()

    with TileContext(nc) as tc:
        with (
            tc.tile_pool(name="const", bufs=1) as constp,
            tc.tile_pool(name="acc", bufs=1) as accp,
            tc.tile_pool(name="scr", bufs=4) as scrp,
            tc.tile_pool(name="se", bufs=6) as sep,
            tc.tile_pool(name="xn", bufs=6) as xnp,
            tc.tile_pool(name="rr", bufs=4) as rrp,
            tc.tile_pool(name="sm", bufs=16) as smp,
            tc.tile_pool(name="psm", bufs=3, space="PSUM") as psm,
            tc.tile_pool(name="pss", bufs=1, space="PSUM") as pss,
            tc.tile_pool(name="psb", bufs=1, space="PSUM") as psb,
        ):
            # ---- inputs resident in SBUF ----
            wt = constp.tile([128, KD, V], FP8, tag="wt")
            for j in range(KD // 2):
                for vh in range(4):
                    vs = slice(vh * (V // 4), (vh + 1) * (V // 4))
                    nc.sync.dma_start(wt[:, 2 * j:2 * j + 2, vs],
                                      WT_d[:, 2 * j:2 * j + 2, vs])
            xt = []
            for bi in range(NB):
                t_ = constp.tile([128, KD, T], FP8, tag=f"xt{bi}", name=f"xt{bi}")
                nc.sync.dma_start(t_[:, :, :], xT_d[bi])
                xt.append(t_)
            ws = []
            for bi in range(NB):
                t_ = constp.tile([128, KD, CB + 1], FP8, tag=f"ws{bi}",
                                 name=f"ws{bi}")
                nc.sync.dma_start(t_[:, :, :], Ws_d[bi])
                ws.append(t_)
            bgs = constp.tile([128, 4], F32, tag="bgs")
            nc.sync.dma_start(bgs[:, :], bg_d)
            bgs2 = constp.tile([128, 4], F32, tag="bgs2")
            nc.sync.dma_start(bgs2[:, :], b2_d)
            bsel = constp.tile([ROWS, 1], F32, tag="bsel")
            nc.sync.dma_start(bsel[:, :], bs_d)
            mk1 = []
            for bi in range(NB):
                t_ = constp.tile([1, T], F32, tag=f"mk{bi}", name=f"mk{bi}")
                nc.sync.dma_start(t_[:, :], mk_d[bi:bi + 1, :])
                mk1.append(t_)
            ind1 = []
            for bi in range(NB):
                t_ = constp.tile([1, ROWS], F32, tag=f"ind{bi}", name=f"ind{bi}")
                nc.sync.dma_start(t_[:, :], in_d[bi:bi + 1, :])
                ind1.append(t_)

            ident = constp.tile([128, 128], F32, tag="ident")
            make_identity(nc, ident[:, :])
            z128 = constp.tile([ROWS, 128], F32, tag="z128")
            nc.vector.memset(z128[:, :], 0.0)
            zrow = constp.tile([1, T], F32, tag="zrow")
            nc.vector.memset(zrow[:, :], 0.0)
            zcol = constp.tile([ROWS, 1], F32, tag="zcol")
            nc.vector.memset(zcol[:, :], 0.0)
            e30 = constp.tile([ROWS, 1], F32, tag="e30")
            nc.vector.memset(e30[:, :], 1e-30)

            # ---- persistent tensors ----
            XN = accp.tile([ROWS, T], F32, tag="XN")   # g rows (candidates)
            AN = accp.tile([ROWS, T], F32, tag="AN")
            SS = accp.tile([ROWS, T], F32, tag="SS")
            FF = accp.tile([ROWS, T], F32, tag="FF")
            BL1 = [accp.tile([1, T], F32, tag=f"BL1_{i}", name=f"BL1_{i}")
                   for i in range(NB)]
            L1 = [accp.tile([1, T], F32, tag=f"L1_{i}", name=f"L1_{i}")
                  for i in range(NB)]
            LM1 = [accp.tile([1, T], F32, tag=f"LM1_{i}", name=f"LM1_{i}")
                   for i in range(NB)]
            LMB = psb.tile([ROWS, T], F32, tag="LMB")

            nc.vector.memset(XN[:, :], NEG)

            mmonly = "mmonly" in variant
            nop2 = "nop2" in variant or mmonly

            q = zcol      # log gauge of AN scale
            p = zcol      # log gauge of SS scale
            negq = zcol
            eqp = None    # exp(q - p), chunks >= 1

            for tcn in range(4):
                tsl = slice(tcn * 128, tcn * 128 + 128)
                for bi in range(NB):
                    se8 = sep.tile([128, 4], F32, tag="se8")
                    for vg in range(4):
                        ps = psm.tile([128, 1024], F32, tag="ps")
                        for h in range(2):
                            vsl = slice((2 * vg + h) * 512, (2 * vg + h) * 512 + 512)
                            for j in range(KD // 2):
                                nc.tensor.matmul(
                                    ps[:, h * 512:h * 512 + 512],
                                    lhsT=xt[bi][:, 2 * j:2 * j + 2, tsl],
                                    rhs=wt[:, 2 * j:2 * j + 2, vsl],
                                    start=(j == 0), stop=(j == KD // 2 - 1),
                                    perf_mode=DR,
                                )
                        if mmonly:
                            continue
                        if vg == 3 and not (tcn == 3 and bi == 3):
                            # Schraudolph bit-trick exp on DVE to offload ACT:
                            # f32(int32(A*l + B)) ~= exp(l); summed for lse.
                            ei = scrp.tile([128, 1024], I32, tag="ei")
                            nc.vector.tensor_scalar(
                                ei[:, :], ps[:, :], SCH_A / WSCALE,
                                bgs2[:, vg:vg + 1], op0=OP.mult, op1=OP.add)
                            nc.vector.tensor_reduce(
                                se8[:, vg:vg + 1], ei[:, :].bitcast(F32),
                                axis=AX, op=OP.add)
                        else:
                            nc.scalar.activation(ps[:, :], ps[:, :], AF.Exp,
                                                 bias=bgs[:, vg:vg + 1],
                                                 scale=1.0 / WSCALE,
                                                 accum_out=se8[:, vg:vg + 1])
                    if mmonly:
                        continue
                    # candidate + blank raw logits (values are 64x here;
                    # exact biases handled via bsel / host L-trend)
                    psl = pss.tile([128, CB + 1], F32, tag="psl")
                    for j in range(KD // 2):
                        nc.tensor.matmul(
                            psl[:, :], lhsT=xt[bi][:, 2 * j:2 * j + 2, tsl],
                            rhs=ws[bi][:, 2 * j:2 * j + 2, :],
                            start=(j == 0), stop=(j == KD // 2 - 1),
                            perf_mode=DR,
                        )
                    se = smp.tile([128, 1], F32, tag="sm")
                    nc.vector.tensor_reduce(se[:, :], se8[:, :], axis=AX, op=OP.add)
                    lse = smp.tile([128, 1], F32, tag="sm")
                    nc.scalar.activation(lse[:, :], se[:, :], AF.Ln)
                    blraw = smp.tile([128, 1], F32, tag="sm")
                    nc.vector.tensor_copy(blraw[:, :], psl[:, 0:1])
                    gsb = xnp.tile([128, 33], F32, tag="gsb")
                    nc.vector.tensor_scalar(gsb[:, 0:CB], psl[:, 1:CB + 1],
                                            blraw[:, :], 1.0 / WSCALE,
                                            op0=OP.subtract, op1=OP.mult)
                    nc.vector.tensor_scalar(gsb[:, 32:33], blraw[:, :],
                                            1.0 / WSCALE, lse[:, :],
                                            op0=OP.mult, op1=OP.subtract)
                    tp = pss.tile([33, 128], F32, tag="psl",
                                  padded_shape=[128, 512])
                    nc.tensor.transpose(tp[:, :], gsb[:, :], ident[:, :])
                    nc.vector.tensor_copy(XN[32 * bi:32 * bi + CB, tsl], tp[0:CB, :])
                    nc.vector.tensor_copy(BL1[bi][:, tsl], tp[32:33, :])

                if nop2:
                    continue

                # ---- phase 2, chunk tcn (overlaps later phase-1 chunks) ----
                nc.vector.tensor_scalar(XN[:, tsl], XN[:, tsl], bsel[:, :], None,
                                        op0=OP.add)
                if tcn == 0:
                    nc.vector.memset(XN[:, 0:4], NEG)
                for bi in range(NB):
                    init = 0.0 if tcn == 0 else L1[bi][:, tcn * 128 - 1:tcn * 128]
                    nc.vector.tensor_tensor_scan(
                        L1[bi][:, tsl], zrow[:, 0:128], BL1[bi][:, tsl], init,
                        op0=OP.add, op1=OP.add)
                    nc.vector.tensor_tensor(LM1[bi][:, tsl], L1[bi][:, tsl],
                                            mk1[bi][:, tsl], op=OP.add)
                    nc.tensor.matmul(LMB[:, tsl], lhsT=ind1[bi][:, :],
                                     rhs=LM1[bi][:, tsl],
                                     start=(bi == 0), stop=(bi == NB - 1))
                r0 = rrp.tile([ROWS, 128], F32, tag="r0")
                nc.scalar.activation(r0[:, :], XN[:, tsl], AF.Exp)
                if tcn > 0:
                    r1 = rrp.tile([ROWS, 128], F32, tag="r1")
                    nc.scalar.activation(r1[:, :], XN[:, tsl], AF.Exp,
                                         bias=negq[:, :])
                else:
                    r1 = r0
                nc.vector.tensor_tensor_scan(AN[:, tsl], r0[:, :], r1[:, :],
                                             0.0 if tcn == 0 else 1.0,
                                             op0=OP.mult, op1=OP.add)
                if tcn > 0:
                    asx = rrp.tile([ROWS, 128], F32, tag="as")
                    nc.vector.tensor_scalar(asx[:, :], AN[:, tsl], eqp[:, :], None,
                                            op0=OP.mult)
                    d1 = asx[:, :]
                else:
                    d1 = AN[:, tsl]
                nc.vector.tensor_tensor_scan(SS[:, tsl], z128[:, :], d1,
                                             0.0 if tcn == 0 else 1.0,
                                             op0=OP.add, op1=OP.add)
                lg = rrp.tile([ROWS, 128], F32, tag="lg")
                nc.scalar.activation(lg[:, :], SS[:, tsl], AF.Ln, bias=e30[:, :])
                nc.vector.scalar_tensor_tensor(FF[:, tsl], lg[:, :], p[:, :],
                                               LMB[:, tsl], op0=OP.add, op1=OP.add)
                if tcn < 3:
                    last = slice(tcn * 128 + 127, tcn * 128 + 128)
                    lq = smp.tile([ROWS, 1], F32, tag="sm2")
                    nc.scalar.activation(lq[:, :], AN[:, last], AF.Ln,
                                         bias=e30[:, :])
                    qn = smp.tile([ROWS, 1], F32, tag="sm2")
                    nc.vector.tensor_tensor(qn[:, :], q[:, :], lq[:, :], op=OP.add)
                    nqn = smp.tile([ROWS, 1], F32, tag="sm2")
                    nc.vector.tensor_scalar(nqn[:, :], qn[:, :], -1.0, None,
                                            op0=OP.mult)
                    ls_ = smp.tile([ROWS, 1], F32, tag="sm2")
                    nc.scalar.activation(ls_[:, :], SS[:, last], AF.Ln,
                                         bias=e30[:, :])
                    pn = smp.tile([ROWS, 1], F32, tag="sm2")
                    nc.vector.tensor_tensor(pn[:, :], p[:, :], ls_[:, :], op=OP.add)
                    dqp = smp.tile([ROWS, 1], F32, tag="sm2")
                    nc.vector.tensor_tensor(dqp[:, :], qn[:, :], pn[:, :],
                                            op=OP.subtract)
                    en = smp.tile([ROWS, 1], F32, tag="sm2")
                    nc.scalar.activation(en[:, :], dqp[:, :], AF.Exp)
                    q, p, negq, eqp = qn, pn, nqn, en

            if nop2:
                cp0 = smp.tile([ROWS, 1], F32, tag="sm2")
                nc.vector.tensor_copy(cp0[:, :], XN[:, 0:1])
                nc.sync.dma_start(cp_d, cp0[:, :])
                for bi in range(NB):
                    nc.sync.dma_start(L_d[bi:bi + 1, :], mk1[bi][:, :])
            else:
                # final masked logsumexp over t (negated max folded into
                # the reduce; curp = Ln(sumexp) - (-max))
                nfm = smp.tile([ROWS, 1], F32, tag="sm2")
                nc.vector.tensor_reduce(nfm[:, :], FF[:, :], axis=AX, op=OP.max,
                                        negate=True)
                trash = scrp.tile([ROWS, T], BF16, tag="trash")
                sF = smp.tile([ROWS, 1], F32, tag="sm2")
                nc.scalar.activation(trash[:, :], FF[:, :], AF.Exp,
                                     bias=nfm[:, :], accum_out=sF[:, :])
                lgs = smp.tile([ROWS, 1], F32, tag="sm2")
                nc.scalar.activation(lgs[:, :], sF[:, :], AF.Ln)
                curp = smp.tile([ROWS, 1], F32, tag="sm2")
                nc.vector.tensor_tensor(curp[:, :], lgs[:, :], nfm[:, :],
                                        op=OP.subtract)
                nc.sync.dma_start(cp_d, curp[:, :])
                for bi in range(NB):
                    nc.sync.dma_start(L_d[bi:bi + 1, :], L1[bi][:, :])

    nc.compile()
    return nc


def _prep_inputs(x, W, b, xl, y, ctc_beam_idx, blank, eos):
    blank = int(blank)
    x = np.asarray(x, np.float32)
    W = np.asarray(W, np.float32)
    b = np.asarray(b, np.float32)
    xl = np.asarray(xl).astype(np.int64)
    idx = np.asarray(ctc_beam_idx).astype(np.int64)

    perm = np.argsort(b, kind="stable")
    Wp = W[perm]                      # vocab sorted by bias; lse is invariant
    bp = b[perm].astype(np.float64)
    WT_np = np.ascontiguousarray(
        (Wp.T * WSCALE).reshape(KD, 128, V).transpose(1, 0, 2)).astype(FP8NP)
    bgs_vals = bp.reshape(4, V // 4).mean(axis=1)    # per 1024-group mean bias
    bgs = np.ascontiguousarray(
        np.broadcast_to(bgs_vals.astype(np.float32), (128, 4)))
    sch_a = 2.0 ** 23 / np.log(2.0)
    bgs2 = np.ascontiguousarray(np.broadcast_to(
        (sch_a * bgs_vals + 1064866805.0).astype(np.float32), (128, 4)))
    ar = np.arange(T)
    in_maps = []
    for c in range(8):
        bs = slice(c * NB, c * NB + NB)
        xb = x[bs]                                            # (NB, T, D)
        xT = np.ascontiguousarray(
            xb.transpose(0, 2, 1).reshape(NB, KD, 128, T).transpose(0, 2, 1, 3)
        ).astype(FP8NP)
        selidx = np.concatenate(
            [np.full((NB, 1), blank, np.int64), idx[bs]], axis=1)  # (NB, 31)
        WselT = np.ascontiguousarray(
            (W[selidx].transpose(0, 2, 1) * WSCALE).reshape(
                NB, KD, 128, CB + 1).transpose(0, 2, 1, 3)).astype(FP8NP)
        bsel = np.zeros((NB, 32), np.float32)
        bsel[:, :CB] = b[idx[bs]] - b[blank]
        bsel = bsel.reshape(ROWS, 1)
        valid = (ar[None, :] >= 4) & (ar[None, :] < xl[bs][:, None])
        mask4 = np.where(valid, 0.0, LOGZERO) + (ar[None, :] + 1) * np.float64(b[blank])
        mask4 = mask4.astype(np.float32)
        ind = np.zeros((NB, ROWS), np.float32)
        for bi in range(NB):
            ind[bi, 32 * bi:32 * bi + CB] = 1.0
        in_maps.append({
            "xT": xT, "WT": WT_np, "WselT": WselT, "bgs": bgs, "bgs2": bgs2,
            "bsel": bsel, "mask4": mask4, "ind": ind,
        })
    return in_maps


def _assemble(results, b, xl, ctc_beam_idx, blank, eos):
    blank = int(blank)
    eos = int(eos)
    b = np.asarray(b, np.float32)
    xl = np.asarray(xl).astype(np.int64)
    idx = np.asarray(ctc_beam_idx).astype(np.int64)
    curP = np.stack(
        [r["curP"].reshape(NB, 32)[:, :CB] for r in results]).reshape(B, CB)
    L = np.stack([r["L"] for r in results]).reshape(B, T)
    L = L + ((np.arange(T) + 1) * np.float64(b[blank])).astype(np.float32)[None, :]

    finalP = np.full((B, V), LOGZERO, np.float32)
    finalP[np.arange(B)[:, None], idx] = curP
    es = np.zeros(B, np.float32)
    ok = (xl >= 1) & (xl <= T)
    if ok.any():
        es[ok] = L[np.arange(B)[ok], (xl[ok] - 1)]
    finalP[:, eos] = es
    finalP[:, blank] = LOGZERO
    return finalP


def kernel(x, W, b, xl, y, ctc_beam_idx, blank, eos):
    nc = _build()
    in_maps = _prep_inputs(x, W, b, xl, y, ctc_beam_idx, blank, eos)
    res = bass_utils.run_bass_kernel_spmd(nc, in_maps, core_ids=list(range(8)))
    return _assemble(res.results, b, xl, ctc_beam_idx, blank, eos)
